# revision 18
# baseline (speedup 1.0000x reference)
"""GCN aggregator kernel for 8 Trainium2 NeuronCores (Bass/Tile).

Computes: out = D_r^{-1/2} M D_c^{-1/2} E[unique_ids]  where M is the
[B, U] 0/1 neighbor mask built from neigh_cols (duplicate (row, col)
pairs collapse to 1).

Sharding (v2 — u-sharded pairs, output ReduceScatter):
  Each core owns u-shard c: u in [4096c, 4096(c+1)). The host buckets
  every (b, k) pair by u >> 12 and hands core c its ~16K pairs packed
  into a [128 lanes x 144 cols] grid where ALL pairs of a given output
  row b sit in one lane (the SWDGE engine assignment is a fixed function
  of pair_pos % 128, so same-row scatter-adds serialize on one DMA
  engine and the read-modify-write is race-free). Duplicate (b, u)
  pairs gather a zero row of E' so they contribute nothing; pad slots
  scatter into a dump row.

Per-core device work:
  1. histogram own pairs' u_local (one-hot matmuls into PSUM[lo, hi];
     host-marked dup/pad slots carry a sentinel whose hi one-hot is all
     zero) -> col counts -> icn = rsqrt(max(cnt, 1)). No collective.
  2. E'[u] = icn[u] * emb[ids[u]] for the own shard ([4096, 128] f32 in
     DRAM, plus a zero row), via 32 indirect row gathers.
  3. 4 chunks: dma_gather pair rows from E' -> dma_scatter_add into a
     [4224, 128] f32 accumulator (rows 0..4095 real, 4096+ dump).
  4. ReduceScatter the [4096, 128] partial outputs (2 MB in, 256 KB out)
     -> each core holds its own 512 output rows.
  5. row norm: rn[b] = rsqrt(#distinct neighbors) from the row-sorted
     neigh_cols (adjacent-compare), multiply, store.
"""

import os
import numpy as np
from contextlib import ExitStack

import concourse.tile as tile
from concourse import bass, bacc, mybir
from concourse.bass_utils import run_bass_kernel_spmd

dt = mybir.dt
Alu = mybir.AluOpType
Act = mybir.ActivationFunctionType

B, K, U, V, D = 4096, 32, 32768, 100000, 128
NC = 8
BC = B // NC            # 512 output rows per core
TPC = BC // 128         # 4 row-tiles per core (b_local = 4p + t)
USH = U // NC           # 4096 unique ids per core
JW = USH // 128         # 32 shard columns (u_local = 128j + p)

LANES = 128
SCALLS = 16             # scatter calls; each has UNIQUE dst rows (the HW
                        # scatter-add RMW races on repeated rows in one call)
SCOL = 9                # grid columns per scatter call
SCAP = LANES * SCOL     # 1152 pair slots per scatter call
GCH = 4                 # gather chunks; each feeds SCALLS/GCH scatter calls
C = SCALLS * SCOL       # 144 grid columns
NP_MAX = LANES * C      # 18432 pair slots
CHIST = 132             # dense histogram grid columns
EROWS = 4224            # E' rows: 4096 real + zero rows (33 * 128)
AROWS = 4224            # accumulator rows: 4096 real + dump region
ZROW = 4096             # zero row in E' (dup/pad gather target)
DUMP = 4096             # dump row in accumulator (dup/pad scatter target)
SENT = 4224             # histogram sentinel: hi = 33 -> one-hot all zero

LAST_RESULTS = None     # test harness reads profiling info from here
_PROGRAM = None


def _build_program():
    skips = set(os.environ.get("GCN_SKIP", "").split(","))
    nc = bacc.Bacc("TRN2", target_bir_lowering=False, debug=False, num_devices=NC)

    t_x = nc.dram_tensor("x", [128, TPC, K], dt.int32, kind="ExternalInput").ap()
    t_hv = nc.dram_tensor("hv", [128, CHIST], dt.int32, kind="ExternalInput").ap()
    t_gidx = nc.dram_tensor("gidx", [128, NP_MAX // 16], dt.int16,
                            kind="ExternalInput").ap()
    t_sidx = nc.dram_tensor("sidx", [128, NP_MAX // 16], dt.int16,
                            kind="ExternalInput").ap()
    t_idst = nc.dram_tensor("idst", [128, JW], dt.int32, kind="ExternalInput").ap()
    t_emb = nc.dram_tensor("emb", [V, D], dt.float32, kind="ExternalInput").ap()
    t_iota = nc.dram_tensor("iotaf", [128, 128], dt.float32,
                            kind="ExternalInput").ap()
    t_out = nc.dram_tensor("out", [BC, D], dt.float32, kind="ExternalOutput").ap()

    # standalone DRAM scratch (offset-0 APs for gather / collectives)
    t_esh = nc.dram_tensor("esh", [EROWS, D], dt.float32).ap()
    t_acc = nc.dram_tensor("acc", [AROWS, D], dt.float32).ap()
    t_rs = nc.dram_tensor("rs", [BC, D], dt.float32).ap()
    debug = int(os.environ.get("GCN_DEBUG", "0"))
    if debug:
        t_dacc = nc.dram_tensor("dacc", [AROWS, D], dt.float32,
                                kind="ExternalOutput").ap()
        t_drs = nc.dram_tensor("drs", [BC, D], dt.float32,
                               kind="ExternalOutput").ap()
    if debug >= 2:
        t_desh = nc.dram_tensor("desh", [EROWS, D], dt.float32,
                                kind="ExternalOutput").ap()
        t_dg = nc.dram_tensor("dg", [GCH, 128, C // GCH, D], dt.float32,
                              kind="ExternalOutput").ap()

    with tile.TileContext(nc) as tc, ExitStack() as ctx:
        sb = ctx.enter_context(tc.tile_pool(name="sb", bufs=1))
        sbd = ctx.enter_context(tc.tile_pool(name="sbd", bufs=4))
        gpool = ctx.enter_context(tc.tile_pool(name="gp", bufs=2))
        ps = ctx.enter_context(tc.tile_pool(name="ps", bufs=1, space="PSUM"))

        # ---------- input loads ----------
        s_x = sb.tile([128, TPC, K], dt.int32)
        nc.sync.dma_start(s_x[:], t_x)
        s_hv = sb.tile([128, CHIST], dt.int32)
        nc.sync.dma_start(s_hv[:], t_hv)
        s_gidx = sb.tile([128, NP_MAX // 16], dt.int16)
        nc.sync.dma_start(s_gidx[:], t_gidx)
        s_sidx = sb.tile([128, NP_MAX // 16], dt.int16)
        nc.sync.dma_start(s_sidx[:], t_sidx)
        s_idst = sb.tile([128, JW], dt.int32)
        nc.sync.dma_start(s_idst[:], t_idst)
        s_iota = sb.tile([128, 128], dt.float32)
        nc.sync.dma_start(s_iota[:], t_iota)

        # ---------- zero the accumulator + E' tail rows ----------
        s_zero = sb.tile([128, AROWS // 128, D], dt.float32)
        nc.vector.memset(s_zero[:], 0.0)
        nc.sync.dma_start(t_acc.rearrange("(g p) d -> p g d", p=128), s_zero[:])
        nc.sync.dma_start(
            t_esh[USH:EROWS, :].rearrange("(g p) d -> p g d", p=128),
            s_zero[:, 0:(EROWS - USH) // 128, :])

        # ---------- E' raw row gather (overlaps histogram) ----------
        s_eb = sb.tile([128, JW, D], dt.float32)
        for j in range(1 if "ebuild" in skips else JW):
            nc.gpsimd.indirect_dma_start(
                out=s_eb[:, j, :], out_offset=None, in_=t_emb,
                in_offset=bass.IndirectOffsetOnAxis(ap=s_idst[:, j:j + 1], axis=0))

        # ---------- histogram of own pairs' u_local ----------
        # u_local = 128*hi + lo ; value SENT has hi = 33 -> not counted.
        s_hvf = sb.tile([128, CHIST], dt.float32)
        nc.vector.tensor_copy(s_hvf[:], s_hv[:])
        s_lo = sb.tile([128, CHIST], dt.int32)
        nc.vector.tensor_scalar(
            out=s_lo[:], in0=s_hv[:], scalar1=127, scalar2=None,
            op0=Alu.bitwise_and)
        s_lof = sb.tile([128, CHIST], dt.float32)
        nc.vector.tensor_copy(s_lof[:], s_lo[:])
        s_hif = sb.tile([128, CHIST], dt.float32)
        nc.vector.tensor_tensor(
            out=s_hif[:], in0=s_hvf[:], in1=s_lof[:], op=Alu.subtract)
        nc.vector.tensor_scalar(
            out=s_hif[:], in0=s_hif[:], scalar1=1.0 / 128.0, scalar2=None,
            op0=Alu.mult)
        s_iob = sb.tile([128, 128], dt.bfloat16)
        nc.vector.tensor_copy(s_iob[:], s_iota[:])

        p_hist = ps.tile([128, JW], dt.float32, space="PSUM")
        nhist = 1 if "hist" in skips else CHIST
        for j in range(nhist):
            lo16 = sbd.tile([128, 128], dt.bfloat16, tag="lo16")
            nc.vector.tensor_scalar(
                out=lo16[:], in0=s_iob[:], scalar1=s_lof[:, j:j + 1],
                scalar2=None, op0=Alu.is_equal)
            hi16 = sbd.tile([128, JW], dt.bfloat16, tag="hi16")
            nc.vector.tensor_scalar(
                out=hi16[:], in0=s_iob[:, 0:JW], scalar1=s_hif[:, j:j + 1],
                scalar2=None, op0=Alu.is_equal)
            nc.tensor.matmul(
                p_hist[:], lhsT=lo16[:], rhs=hi16[:],
                start=(j == 0), stop=(j == nhist - 1))

        # icn[p, j] = rsqrt(max(cnt, 1)) for u_local = 128j + p
        s_icn = sb.tile([128, JW], dt.float32)
        nc.vector.tensor_scalar(
            out=s_icn[:], in0=p_hist[:], scalar1=1.0, scalar2=None, op0=Alu.max)
        nc.scalar.activation(out=s_icn[:], in_=s_icn[:], func=Act.Sqrt)
        nc.vector.reciprocal(out=s_icn[:], in_=s_icn[:])

        # ---------- scale + store E' ----------
        nc.vector.tensor_tensor(
            out=s_eb[:], in0=s_eb[:],
            in1=s_icn[:].to_broadcast([128, JW, D]), op=Alu.mult)
        nc.sync.dma_start(
            t_esh[0:USH, :].rearrange("(j p) d -> p j d", p=128), s_eb[:])

        # ---------- pair gather -> scatter-add ----------
        # Gather chunk ci covers scatter calls [4ci, 4ci+4); each scatter
        # call's 1152 dst rows are unique (host-guaranteed matching).
        giw = NP_MAX // 16 // GCH       # wrapped idx cols per gather chunk
        siw = SCAP // 16                # wrapped idx cols per scatter call
        spc = SCALLS // GCH             # scatter calls per gather chunk
        gcols = C // GCH                # grid cols per gather chunk
        for ci in range(0 if "pairs" in skips else GCH):
            s_g = gpool.tile([128, gcols, D], dt.float32, tag="gch")
            nc.gpsimd.dma_gather(
                out_ap=s_g[:], in_ap=t_esh,
                idxs_ap=s_gidx[:, giw * ci:giw * (ci + 1)],
                num_idxs=LANES * gcols, num_idxs_reg=LANES * gcols,
                elem_size=D, single_packet=False)
            if debug >= 2:
                nc.sync.dma_start(t_dg[ci], s_g[:])
            for sc in range(spc):
                s = spc * ci + sc
                nc.gpsimd.dma_scatter_add(
                    out_ap=t_acc, in_ap=s_g[:, SCOL * sc:SCOL * (sc + 1), :],
                    idxs_ap=s_sidx[:, siw * s:siw * (s + 1)],
                    num_idxs=SCAP, num_idxs_reg=SCAP, elem_size=D)

        # ---------- ReduceScatter the partial outputs ----------
        if "coll" in skips:
            nc.gpsimd.dma_start(t_rs[:], t_acc[0:BC, :])
        else:
            nc.gpsimd.collective_compute(
                "ReduceScatter", Alu.add, replica_groups=[list(range(NC))],
                ins=[t_acc[0:B, :]], outs=[t_rs])

        if debug:
            nc.gpsimd.dma_start(t_dacc[:], t_acc[:])
            nc.gpsimd.dma_start(t_drs[:], t_rs[:])
        if debug >= 2:
            nc.gpsimd.dma_start(t_desh[:], t_esh[:])

        # ---------- row norm from sorted neigh_cols ----------
        s_xf = sb.tile([128, TPC, K], dt.float32)
        nc.vector.tensor_copy(s_xf[:], s_x[:])
        s_ne = sb.tile([128, TPC, K - 1], dt.float32)
        nc.vector.tensor_tensor(
            out=s_ne[:], in0=s_xf[:, :, 1:K], in1=s_xf[:, :, 0:K - 1],
            op=Alu.is_equal)
        s_rc = sb.tile([128, TPC], dt.float32)
        nc.vector.tensor_reduce(
            out=s_rc[:], in_=s_ne[:], axis=mybir.AxisListType.X, op=Alu.add)
        # distinct = K - dups ; rn = 1/sqrt(distinct)
        nc.vector.tensor_scalar(
            out=s_rc[:], in0=s_rc[:], scalar1=-1.0, scalar2=float(K),
            op0=Alu.mult, op1=Alu.add)
        nc.scalar.activation(out=s_rc[:], in_=s_rc[:], func=Act.Sqrt)
        s_rn = sb.tile([128, TPC], dt.float32)
        nc.vector.reciprocal(out=s_rn[:], in_=s_rc[:])

        # ---------- apply rn to own output shard ----------
        s_o = sb.tile([128, TPC, D], dt.float32)
        nc.sync.dma_start(s_o[:], t_rs.rearrange("(p t) d -> p t d", t=TPC))
        nc.vector.tensor_tensor(
            out=s_o[:], in0=s_o[:],
            in1=s_rn[:].to_broadcast([128, TPC, D]), op=Alu.mult)
        nc.sync.dma_start(t_out.rearrange("(p t) d -> p t d", t=TPC), s_o[:])

    nc.compile()
    return nc


def _get_program():
    global _PROGRAM
    if _PROGRAM is None:
        _PROGRAM = _build_program()
    return _PROGRAM


def _wrap_idx(grid):
    """[128, C] grid (pair i = p + 128*col) -> [128, NP//16] wrapped int16."""
    flat = grid.T.reshape(-1)  # linear i = p + 128*col
    w = np.zeros((16, NP_MAX // 16), np.int16)
    i = np.arange(NP_MAX)
    w[i % 16, i // 16] = flat
    return np.tile(w, (8, 1))


def _pack_core(neigh_cols, c):
    """Bucket the pairs owned by u-shard c into SCALLS scatter matchings.

    Each scatter call gets at most one pair per destination row (the HW
    scatter-add is not RMW-safe for repeated rows within one call).
    Balanced wrap-around strip assignment: rows take consecutive call
    slots modulo SCALLS, so loads differ by at most 1.

    Returns (gidx [128,C] i16, sidx [128,C] i16, hv [128,CHIST] i32).
    """
    u = neigh_cols
    sel = (u >> 12) == c
    b_idx, _ = np.nonzero(sel)                 # sorted by (b, k)
    u_c = (u[sel] & 4095).astype(np.int64)
    npair = len(b_idx)

    # dup flag: non-first occurrence of (b, u) — sorted adjacency
    key = (b_idx.astype(np.int64) << 12) | u_c
    order = np.argsort(key, kind="stable")
    ks = key[order]
    dup_sorted = np.zeros(len(ks), bool)
    dup_sorted[1:] = ks[1:] == ks[:-1]
    dup = np.empty_like(dup_sorted)
    dup[order] = dup_sorted

    rb = b_idx[~dup]
    ru = u_c[~dup]
    m_b = np.bincount(rb, minlength=B)         # distinct pairs per row
    if m_b.max() > SCALLS:
        raise RuntimeError(f"core {c}: row multiplicity {m_b.max()} > {SCALLS}")

    # wrap-around strip: row r's j-th pair -> call (start_r + j) % SCALLS
    # (rb is sorted, so j = position within the row's run)
    starts_r = np.zeros(B + 1, np.int64)
    np.cumsum(m_b, out=starts_r[1:])
    j_in_row = np.arange(len(rb)) - starts_r[rb]
    call = (starts_r[rb] + j_in_row) % SCALLS

    # position within each call
    corder = np.argsort(call, kind="stable")
    loads = np.bincount(call, minlength=SCALLS)
    if loads.max() > SCAP:
        raise RuntimeError(f"core {c}: call load {loads.max()} > {SCAP}")
    cstart = np.zeros(SCALLS + 1, np.int64)
    np.cumsum(loads, out=cstart[1:])
    pos = np.empty(len(rb), np.int64)
    pos[corder] = np.arange(len(rb)) - cstart[call[corder]]

    # grid slot: call s occupies columns [SCOL*s, SCOL*(s+1))
    lane = pos % LANES
    col = SCOL * call + pos // LANES
    gidx = np.full((LANES, C), ZROW, np.int64)
    sidx = np.full((LANES, C), DUMP, np.int64)
    gidx[lane, col] = ru
    sidx[lane, col] = rb

    # dense histogram grid: all pairs, dups as sentinel
    hvals = np.where(dup, SENT, u_c)
    hv = np.full(LANES * CHIST, SENT, np.int64)
    hv[:npair] = hvals
    hv = hv.reshape(CHIST, LANES).T            # pair i at [i%128, i//128]

    return (gidx.astype(np.int16), sidx.astype(np.int16),
            np.ascontiguousarray(hv).astype(np.int32))


def _make_in_maps(neigh_cols, unique_ids, embed_table):
    neigh_cols = np.ascontiguousarray(np.asarray(neigh_cols, dtype=np.int32))
    unique_ids = np.ascontiguousarray(np.asarray(unique_ids, dtype=np.int32))
    embed_table = np.ascontiguousarray(np.asarray(embed_table, dtype=np.float32))
    iotaf = np.broadcast_to(np.arange(128, dtype=np.float32), (128, 128)).copy()

    in_maps = []
    for c in range(NC):
        gidx, sidx, hv = _pack_core(neigh_cols, c)
        xs = np.sort(neigh_cols[BC * c:BC * (c + 1)], axis=1)  # row b = 4p+t
        in_maps.append({
            "x": xs.reshape(128, TPC, K),
            "hv": hv,
            "gidx": _wrap_idx(gidx),
            "sidx": _wrap_idx(sidx),
            "idst": unique_ids[USH * c:USH * (c + 1)].reshape(JW, 128).T.copy(),
            "emb": embed_table,
            "iotaf": iotaf,
        })
    return in_maps


def kernel(neigh_cols, unique_ids, embed_table):
    global LAST_RESULTS
    nc = _get_program()
    in_maps = _make_in_maps(neigh_cols, unique_ids, embed_table)
    trace = bool(int(os.environ.get("GCN_TRACE", "0")))
    res = run_bass_kernel_spmd(nc, in_maps, list(range(NC)), trace=trace)
    LAST_RESULTS = res
    out = np.concatenate([res.results[c]["out"] for c in range(NC)], axis=0)
    return out.astype(np.float32)


def bench_exec(inputs, iters=12):
    """Steady-state wall times (us) of the compiled NEFF via a reusable
    sharded jit with device-resident inputs. Excludes compile; includes
    per-call dispatch overhead of the runtime."""
    import time
    import jax
    from jax.sharding import Mesh, PartitionSpec, NamedSharding
    from jax.experimental.shard_map import shard_map
    from concourse.bass2jax import (_bass_exec_p, partition_id_tensor,
                                    install_neuronx_cc_hook)

    nc = _get_program()
    install_neuronx_cc_hook()
    in_maps = _make_in_maps(**inputs)

    partition_name = (nc.partition_id_tensor.name
                      if nc.partition_id_tensor else None)
    in_names, out_names, out_avals, zero_outs = [], [], [], []
    for alloc in nc.m.functions[0].allocations:
        if not isinstance(alloc, mybir.MemoryLocationSet):
            continue
        name = alloc.memorylocations[0].name
        if alloc.kind == "ExternalInput":
            if name != partition_name:
                in_names.append(name)
        elif alloc.kind == "ExternalOutput":
            out_names.append(name)
            shape = tuple(alloc.tensor_shape)
            npdt = dt.np(alloc.dtype)
            out_avals.append(jax.core.ShapedArray(shape, npdt))
            zero_outs.append(np.zeros(shape, npdt))
    n_params = len(in_names)
    all_names = in_names + out_names + ([partition_name] if partition_name else [])

    def _body(*args):
        operands = list(args)
        if partition_name is not None:
            operands.append(partition_id_tensor())
        return tuple(_bass_exec_p.bind(
            *operands, out_avals=tuple(out_avals), in_names=tuple(all_names),
            out_names=tuple(out_names), lowering_input_output_aliases=(),
            sim_require_finite=True, sim_require_nnan=True, nc=nc))

    devices = jax.devices()[:NC]
    mesh = Mesh(np.asarray(devices), ("core",))
    sharded = jax.jit(
        shard_map(_body, mesh=mesh,
                  in_specs=(PartitionSpec("core"),) * (n_params + len(out_names)),
                  out_specs=(PartitionSpec("core"),) * len(out_names),
                  check_rep=False),
        keep_unused=True)
    sh = NamedSharding(mesh, PartitionSpec("core"))
    concat_in = [jax.device_put(
        np.concatenate([np.asarray(in_maps[c][nm]) for c in range(NC)], axis=0),
        sh) for nm in in_names]
    concat_zero = [jax.device_put(
        np.zeros((NC * z.shape[0], *z.shape[1:]), z.dtype), sh)
        for z in zero_outs]
    out = sharded(*concat_in, *concat_zero)
    jax.block_until_ready(out)
    times = []
    for _ in range(iters):
        t0 = time.perf_counter()
        out = sharded(*concat_in, *concat_zero)
        jax.block_until_ready(out)
        times.append((time.perf_counter() - t0) * 1e6)
    return sorted(times)


def modeled_time_ns():
    """Single-core device-occupancy model of the program (cost-model sim)."""
    from concourse.timeline_sim import TimelineSim
    return TimelineSim(_get_program(), trace=False).simulate()
